# revision 17
# baseline (speedup 1.0000x reference)
"""Trainium2 Bass kernel for nn_AugmentedTensor (per-head bilinear form).

out[b,a,o] = sum_{i,j} h0[b,a,i] * h1[b,a,j] * T[a,i,j,o],  h = concat(x, 1)

Single-PE-pass decomposition: fold the j-contraction into PSUM accumulation.
For each j, the moving operand is the Hadamard product
    q_j[i, b] = x0T[i, b] * x1[b, j]
built by DVE as one full-width bf16 tensor_tensor (FD=2048, 2x mode) from a
DMA-broadcast row tile X1B_j[i, b] = x1T[j, b] (host-replicated in DRAM).
PE then accumulates
    out[o, b] += T[a, :, j, :]^T @ q_j        (128 matmuls, 4 PSUM banks)
plus 3 augmentation matmuls for the ones-row/col terms:
    out += T[a,:,128,:]^T @ x0T + T[a,128,:,:]^T @ x1T + tcc^T @ ones.
No per-j-slice small ops anywhere -> no DVE/ACT instruction-overhead wall.
Output leaves transposed [o, b]; host transposes back (free).

Sharding: 8 cores; core c -> head a=c>>1, batch half c&1 (2048 rows).
"""

import numpy as np

BS, A, D, OUT = 4096, 4, 128, 128
NCORES = 8
BH = BS // 2      # batch rows per core
P = 128
NT = BH // P      # 16 b-tiles (for the ACT-offload path)
NJ = 128          # total j
NCH = 6           # ScalarE-offload chunks of 4 j each (j in [0, 24))
NJA = 4 * NCH
NJB = NJ - NJA    # j's on the DMA-broadcast/PE path (j in [24, 128))
NPAIR = NJB // 2  # broadcast DMAs ship j-pairs (8KB descriptors)
NB = 4            # X1B pair-buffer ring depth (8-j lookahead)
NQ = 6            # q ring depth

_CACHE = {}
LAST_RESULT = None


def _split_sync_waits(bir_bytes):
    """The walrus build in this container supports exactly ONE sync-wait per
    instruction; Tile freely emits more. Hoist extra waits onto same-engine
    Nop instructions inserted immediately before the owner (engine streams
    are in-order, so 'wait then instruction' == 'instruction with wait').
    Extra completion-updates (non-DMA only) are hoisted onto following Nops.
    """
    import json

    bir = json.loads(bir_bytes)
    ctr = 0
    for fn in bir.get("functions", []):
        for blk in fn.get("blocks", []):
            ins_list = blk.get("instructions")
            if not ins_list:
                continue
            out = []
            for ins in ins_list:
                si = ins.get("sync_info")
                pre, post = [], []
                if si:
                    waits = si.get("on_wait") or []
                    if len(waits) > 1:
                        for w in waits[:-1]:
                            ctr += 1
                            pre.append({
                                "name": f"WSPLIT-{ctr}",
                                "opcode": "NoOp",
                                "engine": ins["engine"],
                                "debug": ins.get("debug", 0),
                                "ins": [],
                                "outs": [],
                                "sync_info": {"on_update": [], "on_wait": [w]},
                            })
                        si["on_wait"] = [waits[-1]]
                    ups = si.get("on_update") or []
                    if len(ups) > 1:
                        if ins.get("opcode") == "DMACopy":
                            raise RuntimeError(
                                f"DMACopy {ins['name']} has {len(ups)} updates; "
                                "cannot hoist safely")
                        for u in ups[1:]:
                            ctr += 1
                            post.append({
                                "name": f"USPLIT-{ctr}",
                                "opcode": "NoOp",
                                "engine": ins["engine"],
                                "debug": ins.get("debug", 0),
                                "ins": [],
                                "outs": [],
                                "sync_info": {"on_update": [u], "on_wait": []},
                            })
                        si["on_update"] = ups[:1]
                out.extend(pre)
                out.append(ins)
                out.extend(post)
            blk["instructions"] = out
    return json.dumps(bir).encode()


def _install_compile_patch():
    """Route every BIR compile through _split_sync_waits."""
    if _CACHE.get("patched"):
        return
    import concourse.bass_utils as bu

    orig = bu.compile_bir_kernel

    def patched(bir_json, tmpdir, neff_name="file.neff"):
        if isinstance(bir_json, str):
            bir_json = bir_json.encode()
        return orig(_split_sync_waits(bir_json), tmpdir, neff_name)

    bu.compile_bir_kernel = patched
    try:
        import concourse.bass2jax as b2j

        b2j.compile_bir_kernel = patched
    except ImportError:
        pass
    _CACHE["patched"] = True


def _build():
    import concourse.bass as bass
    import concourse.mybir as mybir
    from concourse.tile import TileContext

    f32 = mybir.dt.float32
    bf16 = mybir.dt.bfloat16
    nc = bass.Bass()

    x0t = nc.dram_tensor("x0t", [P, BH], bf16, kind="ExternalInput")
    x1t = nc.dram_tensor("x1t", [P, BH], bf16, kind="ExternalInput")
    x1rep = nc.dram_tensor("x1rep", [NPAIR * P, 2 * BH], bf16,
                           kind="ExternalInput")
    x1n = nc.dram_tensor("x1n", [BH, P], f32, kind="ExternalInput")
    tcm = nc.dram_tensor("tcm", [P, 128 * 128], bf16, kind="ExternalInput")
    tuv = nc.dram_tensor("tuv", [P, 256], bf16, kind="ExternalInput")
    tcc = nc.dram_tensor("tcc", [1, P], bf16, kind="ExternalInput")
    iden = nc.dram_tensor("iden", [P, P], f32, kind="ExternalInput")
    out = nc.dram_tensor("out", [P, BH], f32, kind="ExternalOutput")

    mult = mybir.AluOpType.mult  # noqa
    Copy = mybir.ActivationFunctionType.Copy

    with TileContext(nc) as tc:
        with (
            tc.tile_pool(name="const", bufs=1) as cpool,
            tc.tile_pool(name="xb", bufs=NB) as xpool,
            tc.tile_pool(name="q", bufs=NQ) as qpool,
            tc.tile_pool(name="zr", bufs=2) as zpool,
            tc.tile_pool(name="opsum", bufs=1, space="PSUM") as ppool,
            tc.tile_pool(name="rpsum", bufs=2, space="PSUM") as rpool,
        ):
            # --- resident constants (order = first-consumption order) ---
            tuv_s = cpool.tile([P, 256], bf16, tag="tuv")
            nc.sync.dma_start(tuv_s, tuv[:, :])
            x0t_s = cpool.tile([P, BH], bf16, tag="x0t")
            nc.sync.dma_start(x0t_s, x0t[:, :])
            x1t_s = cpool.tile([P, BH], bf16, tag="x1t")
            nc.sync.dma_start(x1t_s, x1t[:, :])
            tcc_s = cpool.tile([1, P], bf16, tag="tcc")
            nc.sync.dma_start(tcc_s, tcc[:, :])
            ones_s = cpool.tile([1, BH], bf16, tag="ones")
            nc.vector.memset(ones_s, 1.0)
            x1n_s = cpool.tile([P, NT * P], f32, tag="x1n")
            for t in range(NT):
                nc.sync.dma_start(x1n_s[:, t * P : (t + 1) * P],
                                  x1n[t * P : (t + 1) * P, :])
            iden_s = cpool.tile([P, P], f32, tag="iden")
            nc.sync.dma_start(iden_s, iden[:, :])
            # tcm: only the chunks needed at the pipeline head load upfront;
            # the rest stream in just-in-time inside the pair loop.
            tcm_s = cpool.tile([P, 128 * 128], bf16, tag="tcm")

            def load_tcm(g):
                nc.sync.dma_start(tcm_s[:, g * 1024 : (g + 1) * 1024],
                                  tcm[:, g * 1024 : (g + 1) * 1024])

            for g in (3, 0, 1, 2):
                load_tcm(g)

            # one PSUM-resident output, 4 banks of [128o x 512b]
            psm = [ppool.tile([P, 512], f32, tag=f"out{k}", name=f"psm_{k}")
                   for k in range(4)]

            # augmentation terms open each bank's accumulation group
            for k in range(4):
                ksl = slice(k * 512, (k + 1) * 512)
                nc.tensor.matmul(psm[k], tuv_s[:, 0:128], x0t_s[:, ksl],
                                 start=True, stop=False)
                nc.tensor.matmul(psm[k], tuv_s[:, 128:256], x1t_s[:, ksl],
                                 start=False, stop=False)
                nc.tensor.matmul(psm[k], tcc_s, ones_s[:, ksl],
                                 start=False, stop=False)

            def a_chunk(c):
                # ScalarE-offload chunk: j in [4c, 4c+4) for all b-tiles.
                # z-roots fold into psm via one PE transpose per tile.
                jb = 4 * c
                csl = slice(jb * 128, (jb + 4) * 128)
                for t in range(NT):
                    tsl = slice(t * P, (t + 1) * P)
                    k, col = divmod(t, 4)
                    rch = rpool.tile([P, 512], f32, tag="rch",
                                     name=f"rch_{c}_{t}")
                    nc.tensor.matmul(rch, x0t_s[:, tsl], tcm_s[:, csl],
                                     start=True, stop=True)
                    zr = zpool.tile([P, 512], f32, tag="zr",
                                    name=f"zr_{c}_{t}")
                    for jj in range(4):
                        nc.scalar.activation(
                            zr[:, jj * P : (jj + 1) * P],
                            rch[:, jj * P : (jj + 1) * P], Copy,
                            scale=x1n_s[:, t * P + jb + jj : t * P + jb + jj + 1])
                    nc.vector.tensor_add(zr[:, 0:256], zr[:, 0:256],
                                         zr[:, 256:512])
                    nc.vector.tensor_add(zr[:, 0:128], zr[:, 0:128],
                                         zr[:, 128:256])
                    nc.tensor.matmul(psm[k][:, col * P : (col + 1) * P],
                                     zr[:, 0:128], iden_s, is_transpose=True,
                                     start=False, stop=False)

            # pair loop: DMA broadcast (j-pair) -> DVE Hadamard x2 -> PE
            nch = 0
            for jp in range(NPAIR):
                xb = xpool.tile([P, 2 * BH], bf16, tag="xb", name=f"xb_{jp}")
                nc.sync.dma_start(xb, x1rep[jp * P : (jp + 1) * P, :])
                if jp < 12:
                    load_tcm(4 + jp)
                for s in range(2):
                    j = NJA + 2 * jp + s
                    q = qpool.tile([P, BH], bf16, tag="q", name=f"q_{j}")
                    nc.vector.tensor_mul(q, x0t_s,
                                         xb[:, s * BH : (s + 1) * BH])
                    last = j == NJ - 1
                    for k in range(4):
                        ksl = slice(k * 512, (k + 1) * 512)
                        nc.tensor.matmul(psm[k],
                                         tcm_s[:, j * 128 : (j + 1) * 128],
                                         q[:, ksl], start=False, stop=last)
                if jp % 8 == 3 and nch < NCH:
                    a_chunk(nch)
                    nch += 1

            while nch < NCH:
                a_chunk(nch)
                nch += 1

            # drain PSUM -> SBUF (ScalarE path is done by now) -> HBM
            out_sb = cpool.tile([P, BH], f32, tag="out_sb")
            for k in range(4):
                ksl = slice(k * 512, (k + 1) * 512)
                nc.scalar.copy(out_sb[:, ksl], psm[k])
            nc.sync.dma_start(out[:, :], out_sb)

    return nc


def _get_nc():
    if "nc" not in _CACHE:
        _CACHE["nc"] = _build()
    return _CACHE["nc"]


def _make_runner(nc):
    """Persistent sharded-jit runner for the axon/PJRT path (specialized copy
    of bass2jax.run_bass_via_pjrt so repeated calls reuse one compiled
    executable). Returns run(in_maps) -> list[dict[str, np.ndarray]]."""
    import jax
    from jax.sharding import Mesh, PartitionSpec
    from jax.experimental.shard_map import shard_map
    import concourse.mybir as mybir
    from concourse.bass2jax import (
        _bass_exec_p, install_neuronx_cc_hook, partition_id_tensor)

    install_neuronx_cc_hook()

    partition_name = nc.partition_id_tensor.name if nc.partition_id_tensor else None
    in_names, out_names, out_avals, zero_outs = [], [], [], []
    for alloc in nc.m.functions[0].allocations:
        if not isinstance(alloc, mybir.MemoryLocationSet):
            continue
        name = alloc.memorylocations[0].name
        if alloc.kind == "ExternalInput":
            if name != partition_name:
                in_names.append(name)
        elif alloc.kind == "ExternalOutput":
            out_names.append(name)
            shape = tuple(alloc.tensor_shape)
            dtype = mybir.dt.np(alloc.dtype)
            out_avals.append(jax.core.ShapedArray(shape, dtype))
            zero_outs.append(np.zeros(shape, dtype))
    n_params = len(in_names)
    n_outs = len(out_avals)
    all_in_names = list(in_names) + list(out_names)
    if partition_name is not None:
        all_in_names.append(partition_name)
    donate = tuple(range(n_params, n_params + n_outs))

    def _body(*args):
        operands = list(args)
        if partition_name is not None:
            operands.append(partition_id_tensor())
        outs = _bass_exec_p.bind(
            *operands,
            out_avals=tuple(out_avals),
            in_names=tuple(all_in_names),
            out_names=tuple(out_names),
            lowering_input_output_aliases=(),
            sim_require_finite=True,
            sim_require_nnan=True,
            nc=nc,
        )
        return tuple(outs)

    devices = jax.devices()[:NCORES]
    mesh = Mesh(np.asarray(devices), ("core",))
    in_specs = (PartitionSpec("core"),) * (n_params + n_outs)
    out_specs = (PartitionSpec("core"),) * len(out_names)
    sharded = jax.jit(
        shard_map(_body, mesh=mesh, in_specs=in_specs, out_specs=out_specs,
                  check_rep=False),
        donate_argnums=donate, keep_unused=True)

    def run(in_maps, raw=False):
        concat_in = [
            np.concatenate([np.asarray(m[name]) for m in in_maps], axis=0)
            for name in in_names
        ]
        concat_zeros = [
            np.zeros((NCORES * z.shape[0], *z.shape[1:]), z.dtype)
            for z in zero_outs
        ]
        out_arrs = sharded(*concat_in, *concat_zeros)
        if raw:
            return out_arrs
        return [
            {name: np.asarray(out_arrs[i]).reshape(NCORES, *out_avals[i].shape)[c]
             for i, name in enumerate(out_names)}
            for c in range(NCORES)
        ]

    return run


def _run(nc, in_maps):
    """Execute on 8 cores; under axon go through the persistent PJRT runner."""
    from concourse._compat import axon_active

    _install_compile_patch()

    if axon_active():
        if "runner" not in _CACHE:
            _CACHE["runner"] = _make_runner(nc)
        return _CACHE["runner"](in_maps), None

    from concourse.bass_utils import run_bass_kernel_spmd

    res = run_bass_kernel_spmd(nc, in_maps, core_ids=list(range(NCORES)))
    return res.results, res


def _make_in_maps(x0, x1, T):
    import ml_dtypes

    bf16 = ml_dtypes.bfloat16
    x0 = np.asarray(x0, dtype=np.float32)
    x1 = np.asarray(x1, dtype=np.float32)
    T = np.asarray(T, dtype=np.float32)

    in_maps = []
    for c in range(NCORES):
        a, h = divmod(c, 2)
        bsl = slice(h * BH, (h + 1) * BH)
        x0c = np.ascontiguousarray(x0[bsl, a, :])  # (BH, 128)
        x1c = np.ascontiguousarray(x1[bsl, a, :])
        x0T = np.ascontiguousarray(x0c.T).astype(bf16)   # (128, BH)
        x1T = np.ascontiguousarray(x1c.T).astype(bf16)
        # j-pair packed broadcast: row jp*128+p = [x1T[NJA+2jp], x1T[NJA+2jp+1]]
        pairs = x1T[NJA:].reshape(NPAIR, 2 * BH)
        x1rep = np.ascontiguousarray(
            np.broadcast_to(pairs[:, None, :], (NPAIR, P, 2 * BH))
        ).reshape(NPAIR * P, 2 * BH)
        in_maps.append({
            "x0t": x0T,
            "x1t": x1T,
            "x1rep": x1rep,
            "x1n": x1c,
            "iden": np.eye(P, dtype=np.float32),
            "tcm": np.ascontiguousarray(
                T[a, :128, :128, :].reshape(128, 128 * 128)).astype(bf16),
            "tuv": np.ascontiguousarray(
                np.concatenate([T[a, :128, 128, :], T[a, 128, :128, :]],
                               axis=1)).astype(bf16),
            "tcc": np.ascontiguousarray(
                T[a, 128, 128, :].reshape(1, 128)).astype(bf16),
        })
    return in_maps


def kernel(x0, x1, T):
    global LAST_RESULT

    in_maps = _make_in_maps(x0, x1, T)
    nc = _get_nc()
    results, LAST_RESULT = _run(nc, in_maps)

    full = np.empty((BS, A, OUT), dtype=np.float32)
    for c in range(NCORES):
        a, h = divmod(c, 2)
        full[h * BH : (h + 1) * BH, a, :] = results[c]["out"].T
    return full


# revision 18
# speedup vs baseline: 1.4272x; 1.4272x over previous
"""Trainium2 Bass kernel for nn_AugmentedTensor (per-head bilinear form).

out[b,a,o] = sum_{i,j} h0[b,a,i] * h1[b,a,j] * T[a,i,j,o],  h = concat(x, 1)

Single-PE-pass decomposition: fold the j-contraction into PSUM accumulation.
For each j, the moving operand is the Hadamard product
    q_j[i, b] = x0T[i, b] * x1[b, j]
built by DVE as one full-width bf16 tensor_tensor (FD=2048, 2x mode) from a
DMA-broadcast row tile X1B_j[i, b] = x1T[j, b] (host-replicated in DRAM).
PE then accumulates
    out[o, b] += T[a, :, j, :]^T @ q_j        (128 matmuls, 4 PSUM banks)
plus 3 augmentation matmuls for the ones-row/col terms:
    out += T[a,:,128,:]^T @ x0T + T[a,128,:,:]^T @ x1T + tcc^T @ ones.
No per-j-slice small ops anywhere -> no DVE/ACT instruction-overhead wall.
Output leaves transposed [o, b]; host transposes back (free).

Sharding: 8 cores; core c -> head a=c>>1, batch half c&1 (2048 rows).
"""

import numpy as np

BS, A, D, OUT = 4096, 4, 128, 128
NCORES = 8
BH = BS // 2      # batch rows per core
P = 128
NT = BH // P      # 16 b-tiles (for the ACT-offload path)
NJ = 128          # total j
NJA = 20          # j's offloaded to ScalarE (5 chunks of 4, j in [108,128))
NJB = NJ - NJA    # j's on the DMA-broadcast/PE path
NPAIR = NJB // 2  # broadcast DMAs ship j-pairs (8KB descriptors)
NB = 4            # X1B pair-buffer ring depth (8-j lookahead)
NQ = 6            # q ring depth

_CACHE = {}
LAST_RESULT = None


def _split_sync_waits(bir_bytes):
    """The walrus build in this container supports exactly ONE sync-wait per
    instruction; Tile freely emits more. Hoist extra waits onto same-engine
    Nop instructions inserted immediately before the owner (engine streams
    are in-order, so 'wait then instruction' == 'instruction with wait').
    Extra completion-updates (non-DMA only) are hoisted onto following Nops.
    """
    import json

    bir = json.loads(bir_bytes)
    ctr = 0
    for fn in bir.get("functions", []):
        for blk in fn.get("blocks", []):
            ins_list = blk.get("instructions")
            if not ins_list:
                continue
            out = []
            for ins in ins_list:
                si = ins.get("sync_info")
                pre, post = [], []
                if si:
                    waits = si.get("on_wait") or []
                    if len(waits) > 1:
                        for w in waits[:-1]:
                            ctr += 1
                            pre.append({
                                "name": f"WSPLIT-{ctr}",
                                "opcode": "NoOp",
                                "engine": ins["engine"],
                                "debug": ins.get("debug", 0),
                                "ins": [],
                                "outs": [],
                                "sync_info": {"on_update": [], "on_wait": [w]},
                            })
                        si["on_wait"] = [waits[-1]]
                    ups = si.get("on_update") or []
                    if len(ups) > 1:
                        if ins.get("opcode") == "DMACopy":
                            raise RuntimeError(
                                f"DMACopy {ins['name']} has {len(ups)} updates; "
                                "cannot hoist safely")
                        for u in ups[1:]:
                            ctr += 1
                            post.append({
                                "name": f"USPLIT-{ctr}",
                                "opcode": "NoOp",
                                "engine": ins["engine"],
                                "debug": ins.get("debug", 0),
                                "ins": [],
                                "outs": [],
                                "sync_info": {"on_update": [u], "on_wait": []},
                            })
                        si["on_update"] = ups[:1]
                out.extend(pre)
                out.append(ins)
                out.extend(post)
            blk["instructions"] = out
    return json.dumps(bir).encode()


def _install_compile_patch():
    """Route every BIR compile through _split_sync_waits."""
    if _CACHE.get("patched"):
        return
    import concourse.bass_utils as bu

    orig = bu.compile_bir_kernel

    def patched(bir_json, tmpdir, neff_name="file.neff"):
        if isinstance(bir_json, str):
            bir_json = bir_json.encode()
        return orig(_split_sync_waits(bir_json), tmpdir, neff_name)

    bu.compile_bir_kernel = patched
    try:
        import concourse.bass2jax as b2j

        b2j.compile_bir_kernel = patched
    except ImportError:
        pass
    _CACHE["patched"] = True


def _build():
    import concourse.bass as bass
    import concourse.mybir as mybir
    from concourse.tile import TileContext

    f32 = mybir.dt.float32
    bf16 = mybir.dt.bfloat16
    nc = bass.Bass()

    f16 = mybir.dt.float16
    x0t = nc.dram_tensor("x0t", [P, BH], bf16, kind="ExternalInput")
    x1t = nc.dram_tensor("x1t", [P, BH], bf16, kind="ExternalInput")
    x1rep = nc.dram_tensor("x1rep", [NPAIR * P, 2 * BH], bf16,
                       kind="ExternalInput")
    x1n = nc.dram_tensor("x1n", [BH, P], f32, kind="ExternalInput")
    tcm = nc.dram_tensor("tcm", [P, 128 * 128], bf16, kind="ExternalInput")
    tuv = nc.dram_tensor("tuv", [P, 256], bf16, kind="ExternalInput")
    tcc = nc.dram_tensor("tcc", [1, P], bf16, kind="ExternalInput")
    iden = nc.dram_tensor("iden", [P, P], f32, kind="ExternalInput")
    out = nc.dram_tensor("out", [P, BH], f32, kind="ExternalOutput")

    mult = mybir.AluOpType.mult  # noqa
    Copy = mybir.ActivationFunctionType.Copy

    with TileContext(nc) as tc:
        with (
            tc.tile_pool(name="const", bufs=1) as cpool,
            tc.tile_pool(name="xb", bufs=NB) as xpool,
            tc.tile_pool(name="q", bufs=NQ) as qpool,
            tc.tile_pool(name="zr", bufs=2) as zpool,
            tc.tile_pool(name="opsum", bufs=1, space="PSUM") as ppool,
            tc.tile_pool(name="rpsum", bufs=2, space="PSUM") as rpool,
        ):
            # --- resident constants (order = first-consumption order) ---
            tuv_s = cpool.tile([P, 256], bf16, tag="tuv")
            nc.sync.dma_start(tuv_s, tuv[:, :])
            x0t_s = cpool.tile([P, BH], bf16, tag="x0t")
            nc.sync.dma_start(x0t_s, x0t[:, :])
            x1t_s = cpool.tile([P, BH], bf16, tag="x1t")
            nc.sync.dma_start(x1t_s, x1t[:, :])
            tcc_s = cpool.tile([1, P], bf16, tag="tcc")
            nc.sync.dma_start(tcc_s, tcc[:, :])
            ones_s = cpool.tile([1, BH], bf16, tag="ones")
            nc.vector.memset(ones_s, 1.0)
            tcm_s = cpool.tile([P, 128 * 128], bf16, tag="tcm")

            def load_tcm(g):
                nc.sync.dma_start(tcm_s[:, g * 1024 : (g + 1) * 1024],
                                  tcm[:, g * 1024 : (g + 1) * 1024])

            for g in (0, 13, 14, 15):
                load_tcm(g)
            x1n_s = cpool.tile([P, NT * P], f32, tag="x1n")
            for t in range(NT):
                nc.sync.dma_start(x1n_s[:, t * P : (t + 1) * P],
                                  x1n[t * P : (t + 1) * P, :])
            iden_s = cpool.tile([P, P], f32, tag="iden")
            nc.sync.dma_start(iden_s, iden[:, :])

            # one PSUM-resident output, 4 banks of [128o x 512b]
            psm = [ppool.tile([P, 512], f32, tag=f"out{k}", name=f"psm_{k}")
                   for k in range(4)]
            # per-b-tile fp accumulators for the ScalarE-offload j's
            zacc = [cpool.tile([P, P], f32, tag=f"zacc{t}", name=f"zacc_{t}")
                    for t in range(NT)]

            # augmentation terms open each bank's accumulation group
            for k in range(4):
                ksl = slice(k * 512, (k + 1) * 512)
                nc.tensor.matmul(psm[k], tuv_s[:, 0:128], x0t_s[:, ksl],
                                 start=True, stop=False)
                nc.tensor.matmul(psm[k], tuv_s[:, 128:256], x1t_s[:, ksl],
                                 start=False, stop=False)
                nc.tensor.matmul(psm[k], tcc_s, ones_s[:, ksl],
                                 start=False, stop=False)

            def a_chunk(c):
                # ScalarE-offload chunk: j in [NJB+4c, NJB+4c+4) for all tiles
                jb = NJB + 4 * c
                csl = slice(jb * 128, (jb + 4) * 128)
                for t in range(NT):
                    tsl = slice(t * P, (t + 1) * P)
                    rch = rpool.tile([P, 512], f32, tag="rch",
                                     name=f"rch_{c}_{t}")
                    nc.tensor.matmul(rch, x0t_s[:, tsl], tcm_s[:, csl],
                                     start=True, stop=True)
                    zr = zpool.tile([P, 512], bf16, tag="zr",
                                    name=f"zr_{c}_{t}")
                    for jj in range(4):
                        nc.scalar.activation(
                            zr[:, jj * P : (jj + 1) * P],
                            rch[:, jj * P : (jj + 1) * P], Copy,
                            scale=x1n_s[:, t * P + jb + jj : t * P + jb + jj + 1])
                    nc.vector.tensor_add(zr[:, 0:256], zr[:, 0:256],
                                         zr[:, 256:512])
                    nc.vector.tensor_add(zr[:, 0:128], zr[:, 0:128],
                                         zr[:, 128:256])
                    if c == 0:
                        nc.vector.tensor_copy(zacc[t], zr[:, 0:128])
                    else:
                        nc.vector.tensor_add(zacc[t], zacc[t], zr[:, 0:128])

            # pair loop: DMA broadcast (j-pair) -> DVE Hadamard x2 -> PE
            nch = 0
            for jp in range(NPAIR):
                xb = xpool.tile([P, 2 * BH], bf16, tag="xb", name=f"xb_{jp}")
                nc.sync.dma_start(xb, x1rep[jp * P : (jp + 1) * P, :])
                if jp < 12:
                    load_tcm(1 + jp)
                for s in range(2):
                    j = 2 * jp + s
                    q = qpool.tile([P, BH], bf16, tag="q", name=f"q_{j}")
                    nc.vector.tensor_mul(q, x0t_s,
                                         xb[:, s * BH : (s + 1) * BH])
                    for k in range(4):
                        ksl = slice(k * 512, (k + 1) * 512)
                        nc.tensor.matmul(psm[k],
                                         tcm_s[:, j * 128 : (j + 1) * 128],
                                         q[:, ksl], start=False, stop=False)
                if jp % 11 == 5 and nch < 5:
                    a_chunk(nch)
                    nch += 1

            while nch < 5:
                a_chunk(nch)
                nch += 1

            # fold the offload accumulators into the PSUM output (transposed)
            for t in range(NT):
                k, col = divmod(t, 4)
                nc.tensor.matmul(psm[k][:, col * P : (col + 1) * P],
                                 zacc[t], iden_s, is_transpose=True,
                                 start=False, stop=(col == 3))

            # drain PSUM -> SBUF (ScalarE path is done by now) -> HBM
            out_sb = cpool.tile([P, BH], f32, tag="out_sb")
            for k in range(4):
                ksl = slice(k * 512, (k + 1) * 512)
                nc.scalar.copy(out_sb[:, ksl], psm[k])
            nc.sync.dma_start(out[:, :], out_sb)

    return nc


def _get_nc():
    if "nc" not in _CACHE:
        _CACHE["nc"] = _build()
    return _CACHE["nc"]


def _make_runner(nc):
    """Persistent sharded-jit runner for the axon/PJRT path (specialized copy
    of bass2jax.run_bass_via_pjrt so repeated calls reuse one compiled
    executable). Returns run(in_maps) -> list[dict[str, np.ndarray]]."""
    import jax
    from jax.sharding import Mesh, PartitionSpec
    from jax.experimental.shard_map import shard_map
    import concourse.mybir as mybir
    from concourse.bass2jax import (
        _bass_exec_p, install_neuronx_cc_hook, partition_id_tensor)

    install_neuronx_cc_hook()

    partition_name = nc.partition_id_tensor.name if nc.partition_id_tensor else None
    in_names, out_names, out_avals, zero_outs = [], [], [], []
    for alloc in nc.m.functions[0].allocations:
        if not isinstance(alloc, mybir.MemoryLocationSet):
            continue
        name = alloc.memorylocations[0].name
        if alloc.kind == "ExternalInput":
            if name != partition_name:
                in_names.append(name)
        elif alloc.kind == "ExternalOutput":
            out_names.append(name)
            shape = tuple(alloc.tensor_shape)
            dtype = mybir.dt.np(alloc.dtype)
            out_avals.append(jax.core.ShapedArray(shape, dtype))
            zero_outs.append(np.zeros(shape, dtype))
    n_params = len(in_names)
    n_outs = len(out_avals)
    all_in_names = list(in_names) + list(out_names)
    if partition_name is not None:
        all_in_names.append(partition_name)
    donate = tuple(range(n_params, n_params + n_outs))

    def _body(*args):
        operands = list(args)
        if partition_name is not None:
            operands.append(partition_id_tensor())
        outs = _bass_exec_p.bind(
            *operands,
            out_avals=tuple(out_avals),
            in_names=tuple(all_in_names),
            out_names=tuple(out_names),
            lowering_input_output_aliases=(),
            sim_require_finite=True,
            sim_require_nnan=True,
            nc=nc,
        )
        return tuple(outs)

    devices = jax.devices()[:NCORES]
    mesh = Mesh(np.asarray(devices), ("core",))
    in_specs = (PartitionSpec("core"),) * (n_params + n_outs)
    out_specs = (PartitionSpec("core"),) * len(out_names)
    sharded = jax.jit(
        shard_map(_body, mesh=mesh, in_specs=in_specs, out_specs=out_specs,
                  check_rep=False),
        donate_argnums=donate, keep_unused=True)

    def run(in_maps, raw=False):
        concat_in = [
            np.concatenate([np.asarray(m[name]) for m in in_maps], axis=0)
            for name in in_names
        ]
        concat_zeros = [
            np.zeros((NCORES * z.shape[0], *z.shape[1:]), z.dtype)
            for z in zero_outs
        ]
        out_arrs = sharded(*concat_in, *concat_zeros)
        if raw:
            return out_arrs
        return [
            {name: np.asarray(out_arrs[i]).reshape(NCORES, *out_avals[i].shape)[c]
             for i, name in enumerate(out_names)}
            for c in range(NCORES)
        ]

    return run


def _run(nc, in_maps):
    """Execute on 8 cores; under axon go through the persistent PJRT runner."""
    from concourse._compat import axon_active

    _install_compile_patch()

    if axon_active():
        if "runner" not in _CACHE:
            _CACHE["runner"] = _make_runner(nc)
        return _CACHE["runner"](in_maps), None

    from concourse.bass_utils import run_bass_kernel_spmd

    res = run_bass_kernel_spmd(nc, in_maps, core_ids=list(range(NCORES)))
    return res.results, res


def _make_in_maps(x0, x1, T):
    import ml_dtypes

    bf16 = ml_dtypes.bfloat16
    x0 = np.asarray(x0, dtype=np.float32)
    x1 = np.asarray(x1, dtype=np.float32)
    T = np.asarray(T, dtype=np.float32)

    in_maps = []
    for c in range(NCORES):
        a, h = divmod(c, 2)
        bsl = slice(h * BH, (h + 1) * BH)
        x0c = np.ascontiguousarray(x0[bsl, a, :])  # (BH, 128)
        x1c = np.ascontiguousarray(x1[bsl, a, :])
        x0T = np.ascontiguousarray(x0c.T).astype(bf16)   # (128, BH)
        x1T = np.ascontiguousarray(x1c.T).astype(bf16)
        pairs = x1T[:NJB].reshape(NPAIR, 2 * BH)
        x1rep = np.ascontiguousarray(
            np.broadcast_to(pairs[:, None, :], (NPAIR, P, 2 * BH))
        ).reshape(NPAIR * P, 2 * BH)
        in_maps.append({
            "x0t": x0T,
            "x1t": x1T,
            "x1rep": x1rep,
            "x1n": x1c,
            "iden": np.eye(P, dtype=np.float32),
            "tcm": np.ascontiguousarray(
                T[a, :128, :128, :].reshape(128, 128 * 128)).astype(bf16),
            "tuv": np.ascontiguousarray(
                np.concatenate([T[a, :128, 128, :], T[a, 128, :128, :]],
                               axis=1)).astype(bf16),
            "tcc": np.ascontiguousarray(
                T[a, 128, 128, :].reshape(1, 128)).astype(bf16),
        })
    return in_maps


def kernel(x0, x1, T):
    global LAST_RESULT

    in_maps = _make_in_maps(x0, x1, T)
    nc = _get_nc()
    results, LAST_RESULT = _run(nc, in_maps)

    full = np.empty((BS, A, OUT), dtype=np.float32)
    for c in range(NCORES):
        a, h = divmod(c, 2)
        full[h * BH : (h + 1) * BH, a, :] = results[c]["out"].T
    return full
